# revision 39
# baseline (speedup 1.0000x reference)
"""Trainium2 Bass kernel for Restormer-style transposed (channel) attention.

Math (per batch b):
  qkv = qkv_w @ x                    (1x1 conv, channel GEMM)
  qkv = DWConv3x3(qkv)               (per-channel 3x3, SAME zero pad)
  q,k,v = split(qkv); per head: q,k l2-normalized over spatial
  attn  = softmax((q_n @ k_n^T) * temperature)
  y     = proj_w @ (blockdiag(attn) @ v)

Sharding: 4 cores <-> one batch image each (no cross-core reduction, so no
collective — its per-call comm setup cost ~0.4s dominated the old 8-core
variant). The final projection is folded with the attention:
y = (proj_w @ blockdiag(attn)) @ v, one 192x192 GEMM applied to v.

v3 layout (program-size-optimized, ~1300 BIR instructions):
  - hardware For_i loops (staggered reset) over row chunks; dynamic DRAM
    offsets via one stride-stepped IV (vres/y padded to the x chunk stride)
  - depthwise conv reads shifted windows directly (odd bf16 elem offsets OK)
  - conv taps: DVE 7 taps as tensor_scalar(4x) + tensor_tensor-add(2x)
    pairs; ACT 2 products via activation(Copy, scale=w); Pool adds + merge
    (Pool has no tensor_scalar opcode and no PSUM port on real TRN2)
  - merged q/k transposes: one [128,384] bf16 PSUM tile, 3 transposes per
    128-px block + 2 head-diagonal Gram matmuls ([96,96] each); Gram
    accumulated in PSUM per chunk, drained to SBUF f32
  - y output in bf16 (host upcasts), halves transfer volume

Engine layout per core:
  PE : qkv GEMM, q/k transposes, Gram, post matmuls, final GEMM
  DVE: conv taps (7/9), merge drains, softmax small ops, y2 evac
  Pool: conv partial adds + deferred merges
  ACT: PSUM->SBUF evacuations, 2 conv products, sumsq (Square+accum), exp
"""

import sys
sys.path.insert(0, '/opt/trn_rl_repo')

import numpy as np
import ml_dtypes
from contextlib import ExitStack

import concourse.bass as bass
import concourse.bacc as bacc
import concourse.tile as tile
import concourse.mybir as mybir
from concourse.bass_utils import run_bass_kernel_spmd

bf16 = mybir.dt.bfloat16
f32 = mybir.dt.float32
Alu = mybir.AluOpType
Act = mybir.ActivationFunctionType

B, C, HEADS, CPH = 4, 192, 4, 48
W = 256
WP = W + 4          # 2 zero cols left, 2 right
N_CORES = 4

# o-tile partition map over the 576 qkv channels (q:0-191, k:192-383, v:384-575)
OT = [(0, 128), (128, 128), (256, 128), (384, 128), (512, 64)]
# conv tap split (Pool has no tensor_scalar / PSUM on HW):
#   DVE: 7 taps as tensor_scalar(4x) + tensor_tensor add(2x) pairs -> co
#   ACT: 2 tap products via activation(Copy, scale=w) -> cp, tma
#   Pool: tensor_tensor adds: cp += tma, and deferred merge co += cp
DVE_TAPS = [(0, 0), (0, -1), (0, 1), (-1, 0), (1, 0), (-1, -1), (1, -1)]
ACT_TAPS = [(-1, 1), (1, 1)]
TAP_IDX = {t: i for i, t in enumerate(
    [(dy, dx) for dy in (-1, 0, 1) for dx in (-1, 0, 1)])}

_BUILT = {}


def build(H=256, CR=16, unroll=False):
    """H: image height (256 real). CR: valid rows per chunk."""
    HALF = H                           # image rows per core (full image)
    NCH = HALF // CR                   # chunks
    assert NCH * CR == HALF
    SH_ROWS = HALF + 2                 # shard rows incl. conv halo
    NF = (CR + 2) * WP                 # GEMM window elems per chunk
    NV = CR * W                        # valid conv-out elems per chunk
    NPX = HALF * W                     # valid pixels per core
    PXB = NV // 128                    # 128-px blocks per chunk
    XST = CR * WP                      # chunk stride in x / vres / y layouts

    nc = bacc.Bacc("TRN2", target_bir_lowering=False, debug=False,
                   num_devices=N_CORES)
    dram = lambda n, s, d, kind: nc.dram_tensor(n, s, d, kind=kind).ap()
    x_d = dram("x", [C, SH_ROWS * WP], bf16, "ExternalInput")
    wt1_d = dram("wt1", [128, 576], bf16, "ExternalInput")
    wt2_d = dram("wt2", [64, 576], bf16, "ExternalInput")
    wdw_d = dram("wdw", [128, 45], f32, "ExternalInput")
    pjt1_d = dram("pjt1", [128, 192], bf16, "ExternalInput")
    pjt2_d = dram("pjt2", [64, 192], bf16, "ExternalInput")
    id_d = dram("ident", [128, 128], bf16, "ExternalInput")
    tmp_d = dram("tempb", [48, 4], f32, "ExternalInput")
    # y uses the x chunk stride (XST >= NV) so one For_i IV addresses both
    y_d = dram("y", [C, NCH * XST], bf16, "ExternalOutput")
    dbg_d = dram("dbg", [128, 200], f32, "ExternalOutput")

    with tile.TileContext(nc) as tc, ExitStack() as ctx:
        P = lambda name, bufs, space="SBUF": ctx.enter_context(
            tc.tile_pool(name=name, bufs=bufs, space=space))
        wp = P("wp", 1)
        xp = P("xp", 1)
        qkvp = P("qkvp", 3)
        cop = P("cop", 2)      # per-o-tile tags co0..co2, cov
        cpp = P("cpp", 2)      # ACT/Pool-side conv accumulator
        tmap = P("tmap", 2)    # ACT tap product staging
        tmdp = P("tmdp", 1)    # DVE tap product staging
        sqp = P("sqp", 2)
        qtp = P("qtp", 2)
        stp = P("stp", 1)
        postp = P("postp", 1)
        vcp = P("vcp", 1)
        ysp = P("ysp", 1)
        ps_g = P("ps_g", 2, "PSUM")
        ps_t = P("ps_t", 2, "PSUM")
        ps_gram = P("ps_gram", 1, "PSUM")
        ps_post = P("ps_post", 2, "PSUM")
        drp = P("drp", 1, "DRAM")

        # ---- weights / constants in SBUF
        wt1 = wp.tile([128, 576], bf16, tag="wt1")
        wt2 = wp.tile([64, 576], bf16, tag="wt2")
        wdw = wp.tile([128, 45], f32, tag="wdw")
        pjt1 = wp.tile([128, 192], bf16, tag="pjt1")
        pjt2 = wp.tile([64, 192], bf16, tag="pjt2")
        ident = wp.tile([128, 128], bf16, tag="ident")
        tempb = wp.tile([48, 4], f32, tag="tempb")
        for t, d in [(wt1, wt1_d), (wt2, wt2_d), (wdw, wdw_d), (pjt1, pjt1_d),
                     (pjt2, pjt2_d), (ident, id_d), (tempb, tmp_d)]:
            nc.sync.dma_start(t[:], d[:])

        # persistent accumulators (f32, SBUF)
        ss_tot = stp.tile([128, 3], f32, tag="ss")        # sumsq per q/k o-tile
        gacc1 = stp.tile([96, 96], f32, tag="gacc1")      # Gram heads 0,1
        gacc2 = stp.tile([96, 96], f32, tag="gacc2")      # Gram heads 2,3
        stats = stp.tile([128, 200], f32, tag="stats")
        stats_rd = stp.tile([128, 200], f32, tag="stats_rd")
        vres = drp.tile([C, NCH * XST], bf16, tag="vres")

        nc.vector.memset(ss_tot[:], 0.0)
        nc.vector.memset(gacc1[:], 0.0)
        nc.vector.memset(gacc2[:], 0.0)
        nc.vector.memset(stats[:], 0.0)

        # ================= main chunk loop =================
        def chunk_body(xoff, shift):
            x1v = x_d[0:128, shift:]
            x2v = x_d[128:192, shift:]
            xc1 = xp.tile([128, NF], bf16, tag="xc1")
            xc2 = xp.tile([64, NF], bf16, tag="xc2")
            nc.sync.dma_start(xc1[:], x1v[:, bass.ds(xoff, NF)])
            nc.sync.dma_start(xc2[:], x2v[:, bass.ds(xoff, NF)])

            cos = []
            pend = []

            def flush(item):
                j, o0, orows, co, cp, qk = item
                # merge ACT/Pool partial into co (Pool), deferred two o-tiles
                # so no engine stalls waiting on another chain
                nc.gpsimd.tensor_tensor(co[0:orows, :], co[0:orows, :],
                                        cp[0:orows, :], Alu.add)
                if j >= 3:   # v channels -> DRAM scratch
                    vv = vres[o0 - 384:o0 - 384 + orows, shift:]
                    nc.sync.dma_start(vv[:, bass.ds(xoff, NV)], co[0:orows, :])
                else:        # q/k channels: sumsq partial (ACT) + keep for T
                    cos.append(co)
                    sqt = sqp.tile([128, 1], f32, tag="sqt")
                    # Square's bulk output goes into qk's buffer (fully consumed)
                    nc.scalar.activation(
                        qk[0:orows, 0:NV], co[0:orows, :], Act.Square,
                        accum_out=sqt[0:orows, 0:1])
                    nc.vector.tensor_tensor(
                        ss_tot[0:orows, j:j + 1], ss_tot[0:orows, j:j + 1],
                        sqt[0:orows, 0:1], Alu.add)

            for j, (o0, orows) in enumerate(OT):
                # --- GEMM into psum pieces, evac to SBUF (ACT)
                qk = qkvp.tile([128, NF], bf16, tag="qk")
                p = 0
                while p < NF:
                    pw = min(512, NF - p)
                    ps = ps_g.tile([128, 512], f32, tag="gemm")
                    nc.tensor.matmul(ps[0:orows, 0:pw],
                                     wt1[:, o0:o0 + orows],
                                     xc1[:, p:p + pw], start=True, stop=False)
                    nc.tensor.matmul(ps[0:orows, 0:pw],
                                     wt2[:, o0:o0 + orows],
                                     xc2[:, p:p + pw], start=False, stop=True)
                    nc.scalar.copy(qk[0:orows, p:p + pw], ps[0:orows, 0:pw])
                    p += pw

                # --- depthwise 3x3 direct from qk (odd elem offsets are OK)
                co = cop.tile([128, NV], bf16, tag=f"co{j}" if j < 3 else "cov")
                cp = cpp.tile([128, NV], bf16, tag="cp")
                co3 = co[0:orows, :].rearrange("p (r c) -> p r c", c=W)
                cp3 = cp[0:orows, :].rearrange("p (r c) -> p r c", c=W)
                qk3 = qk[0:orows, :].rearrange("p (r c) -> p r c", c=WP)
                wofs = 9 * j

                def src(dy, dx, r0=0, r1=CR):
                    return qk3[:, 1 + dy + r0:1 + dy + r1, 2 + dx:2 + dx + W]

                def wsl(dy, dx):
                    i = wofs + TAP_IDX[(dy, dx)]
                    return wdw[0:orows, i:i + 1]

                # ACT: two tap products (per-partition scale); Pool adds them
                tma = tmap.tile([128, NV], bf16, tag="tma")
                tma3 = tma[0:orows, :].rearrange("p (r c) -> p r c", c=W)
                (dy0, dx0), (dy1, dx1) = ACT_TAPS
                nc.scalar.activation(cp3[:], src(dy0, dx0), Act.Copy,
                                     scale=wsl(dy0, dx0))
                nc.scalar.activation(tma3[:], src(dy1, dx1), Act.Copy,
                                     scale=wsl(dy1, dx1))
                nc.gpsimd.tensor_tensor(cp[0:orows, :], cp[0:orows, :],
                                        tma[0:orows, :], Alu.add)
                # DVE: 7 taps as 4x tensor_scalar + 2x tensor_tensor-add pairs
                tmd = tmdp.tile([128, NV], bf16, tag="tmd")
                tmd3 = tmd[0:orows, :].rearrange("p (r c) -> p r c", c=W)
                for t, (dy, dx) in enumerate(DVE_TAPS):
                    if t == 0:
                        nc.vector.tensor_scalar(co3[:], src(dy, dx),
                                                wsl(dy, dx), None, Alu.mult)
                    else:
                        nc.vector.tensor_scalar(tmd3[:], src(dy, dx),
                                                wsl(dy, dx), None, Alu.mult)
                        nc.vector.tensor_tensor(co[0:orows, :], co[0:orows, :],
                                                tmd[0:orows, :], Alu.add)
                pend.append((j, o0, orows, co, cp, qk))
                if len(pend) > 2:
                    flush(pend.pop(0))
            while pend:
                flush(pend.pop(0))
            return cos

        def tg_body(cos):
            # --- transposes + Gram over one chunk's q/k
            co0, co1, co2 = cos
            for bb in range(PXB):
                blk = bb * 128
                t_ps = ps_t.tile([128, 384], bf16, tag="tps")
                nc.tensor.transpose(t_ps[:, 0:128], co0[:, blk:blk + 128],
                                    ident[:])
                nc.tensor.transpose(t_ps[:, 128:256], co1[:, blk:blk + 128],
                                    ident[:])
                nc.tensor.transpose(t_ps[:, 256:384], co2[:, blk:blk + 128],
                                    ident[:])
                t_sb = qtp.tile([128, 384], bf16, tag="tsb")
                nc.scalar.copy(t_sb[:], t_ps[:])
                first, last = (bb == 0), (bb == PXB - 1)
                nc.tensor.matmul(g1_ps[:], t_sb[:, 0:96], t_sb[:, 192:288],
                                 start=first, stop=last)
                nc.tensor.matmul(g2_ps[:], t_sb[:, 96:192], t_sb[:, 288:384],
                                 start=first, stop=last)
            # drain chunk Gram into SBUF accumulators
            nc.vector.tensor_tensor(gacc1[:], gacc1[:], g1_ps[:], Alu.add)
            nc.vector.tensor_tensor(gacc2[:], gacc2[:], g2_ps[:], Alu.add)

        g1_ps = ps_gram.tile([96, 96], f32, tag="g1")
        g2_ps = ps_gram.tile([96, 96], f32, tag="g2")
        if unroll:
            for ci in range(NCH):
                tg_body(chunk_body(ci * XST, 0))
        else:
            with tc.For_i(0, NCH * XST, XST, staggered_reset=True,
                          hint_engines=(mybir.EngineType.PE,)) as xoff:
                tg_body(chunk_body(xoff, 0))

        # ================= stats pack + AllReduce =================
        # Gram diagonal blocks -> stats[0:48, 48h:48h+48]
        nc.sync.dma_start(stats[0:48, 0:48], gacc1[0:48, 0:48])
        nc.sync.dma_start(stats[0:48, 48:96], gacc1[48:96, 48:96])
        nc.sync.dma_start(stats[0:48, 96:144], gacc2[0:48, 0:48])
        nc.sync.dma_start(stats[0:48, 144:192], gacc2[48:96, 48:96])
        for j in range(3):
            nc.vector.tensor_copy(stats[0:OT[j][1], 192 + j:193 + j],
                                  ss_tot[0:OT[j][1], j:j + 1])
        nc.vector.tensor_copy(stats_rd[:], stats[:])
        nc.sync.dma_start(dbg_d[:], stats_rd[:])

        # ================= softmax(attn) =================
        # reassemble qss/kss as [48 part, 4 head] via partition-moving DMAs
        qss = postp.tile([48, 4], f32, tag="qss")
        kss = postp.tile([48, 4], f32, tag="kss")
        mv = [
            (qss, 0, 0, 48, 192, 0), (qss, 1, 0, 48, 192, 48),
            (qss, 2, 0, 32, 192, 96), (qss, 2, 32, 16, 193, 0),
            (qss, 3, 0, 48, 193, 16),
            (kss, 0, 0, 48, 193, 64), (kss, 1, 0, 16, 193, 112),
            (kss, 1, 16, 32, 194, 0), (kss, 2, 0, 48, 194, 32),
            (kss, 3, 0, 48, 194, 80),
        ]
        for dst, h, dp, n, col, sp in mv:
            nc.sync.dma_start(dst[dp:dp + n, h:h + 1],
                              stats_rd[sp:sp + n, col:col + 1])
        rq = postp.tile([48, 4], f32, tag="rq")
        rk = postp.tile([48, 4], f32, tag="rk")
        nc.scalar.sqrt(rq[:], qss[:])
        nc.scalar.sqrt(rk[:], kss[:])
        nc.vector.reciprocal(rq[:], rq[:])
        nc.vector.reciprocal(rk[:], rk[:])
        nc.vector.tensor_tensor(rq[:], rq[:], tempb[:], Alu.mult)
        # rk as a [1,192] row (h-major): dst free idx 48h+d
        rk_row = postp.tile([1, 192], f32, tag="rkrow")
        for h in range(4):
            nc.sync.dma_start(rk_row[0:1, 48 * h:48 * h + 48], rk[:, h:h + 1])
        rk_row_b = postp.tile([1, 192], bf16, tag="rkrowb")
        nc.vector.tensor_copy(rk_row_b[:], rk_row[:])
        ones_f = postp.tile([1, 48], bf16, tag="onesf")
        nc.vector.memset(ones_f[:], 1.0)
        rk_bc = ps_post.tile([48, 192], f32, tag="post")
        nc.tensor.matmul(rk_bc[:], ones_f[:], rk_row_b[:],
                         start=True, stop=True)
        logits = postp.tile([48, 192], f32, tag="logits")
        for h in range(4):
            sl = slice(48 * h, 48 * h + 48)
            nc.vector.tensor_scalar(logits[:, sl], stats_rd[0:48, sl],
                                    rq[:, h:h + 1], None, Alu.mult)
        nc.vector.tensor_tensor(logits[:], logits[:], rk_bc[:], Alu.mult)
        l3 = logits[:].rearrange("p (h d) -> p h d", h=4)
        rmax = postp.tile([48, 4], f32, tag="rmax")
        nc.vector.tensor_reduce(rmax[:], l3, mybir.AxisListType.X, Alu.max)
        for h in range(4):
            sl = slice(48 * h, 48 * h + 48)
            nc.vector.tensor_scalar(logits[:, sl], logits[:, sl],
                                    rmax[:, h:h + 1], None, Alu.subtract)
        nc.scalar.activation(logits[:], logits[:], Act.Exp)
        rsum = postp.tile([48, 4], f32, tag="rsum")
        nc.vector.tensor_reduce(rsum[:], l3, mybir.AxisListType.X, Alu.add)
        nc.vector.reciprocal(rsum[:], rsum[:])
        attn = postp.tile([48, 192], bf16, tag="attn")
        for h in range(4):
            sl = slice(48 * h, 48 * h + 48)
            nc.vector.tensor_scalar(attn[:, sl], logits[:, sl],
                                    rsum[:, h:h + 1], None, Alu.mult)

        # ================= M_bT = BD^T @ projT =================
        bd1 = postp.tile([128, 192], bf16, tag="bd1")
        bd2 = postp.tile([64, 192], bf16, tag="bd2")
        nc.vector.memset(bd1[:], 0.0)
        nc.vector.memset(bd2[:], 0.0)
        nc.sync.dma_start(bd1[0:48, 0:48], attn[:, 0:48])
        nc.sync.dma_start(bd1[48:96, 48:96], attn[:, 48:96])
        nc.sync.dma_start(bd1[96:128, 96:144], attn[0:32, 96:144])
        nc.sync.dma_start(bd2[0:16, 96:144], attn[32:48, 96:144])
        nc.sync.dma_start(bd2[16:64, 144:192], attn[:, 144:192])
        mbt_ps1 = ps_post.tile([128, 192], f32, tag="post")
        nc.tensor.matmul(mbt_ps1[:], bd1[:, 0:128], pjt1[:], start=True, stop=False)
        nc.tensor.matmul(mbt_ps1[:], bd2[:, 0:128], pjt2[:], start=False, stop=True)
        mbt1 = postp.tile([128, 192], bf16, tag="mbt1")
        nc.scalar.copy(mbt1[:], mbt_ps1[:])
        mbt_ps2 = ps_post.tile([64, 192], f32, tag="post")
        nc.tensor.matmul(mbt_ps2[:], bd1[:, 128:192], pjt1[:], start=True, stop=False)
        nc.tensor.matmul(mbt_ps2[:], bd2[:, 128:192], pjt2[:], start=False, stop=True)
        mbt2 = postp.tile([64, 192], bf16, tag="mbt2")
        nc.scalar.copy(mbt2[:], mbt_ps2[:])

        # ================= y = M_b @ v =================
        def final_body(voff, shift):
            v1v = vres[0:128, shift:]
            v2v = vres[128:192, shift:]
            y1v = y_d[0:128, shift:]
            y2v = y_d[128:192, shift:]
            v1 = vcp.tile([128, NV], bf16, tag="v1")
            v2 = vcp.tile([64, NV], bf16, tag="v2")
            nc.sync.dma_start(v1[:], v1v[:, bass.ds(voff, NV)])
            nc.sync.dma_start(v2[:], v2v[:, bass.ds(voff, NV)])
            y1 = ysp.tile([128, NV], bf16, tag="y1")
            y2 = ysp.tile([64, NV], bf16, tag="y2")
            for p in range(0, NV, 512):
                ps = ps_g.tile([128, 512], f32, tag="gemm")
                nc.tensor.matmul(ps[:], mbt1[:, 0:128], v1[:, p:p + 512],
                                 start=True, stop=False)
                nc.tensor.matmul(ps[:], mbt2[:, 0:128], v2[:, p:p + 512],
                                 start=False, stop=True)
                nc.scalar.copy(y1[:, p:p + 512], ps[:])
                ps2 = ps_g.tile([128, 512], f32, tag="gemm")
                nc.tensor.matmul(ps2[0:64, :], mbt1[:, 128:192], v1[:, p:p + 512],
                                 start=True, stop=False)
                nc.tensor.matmul(ps2[0:64, :], mbt2[:, 128:192], v2[:, p:p + 512],
                                 start=False, stop=True)
                nc.vector.tensor_copy(y2[:, p:p + 512], ps2[0:64, :])
            nc.sync.dma_start(y1v[:, bass.ds(voff, NV)], y1[:])
            nc.sync.dma_start(y2v[:, bass.ds(voff, NV)], y2[:])

        if unroll:
            for vi in range(NCH):
                final_body(vi * XST, 0)
        else:
            with tc.For_i(0, NCH * XST, XST, staggered_reset=True) as voff:
                final_body(voff, 0)

    nc.compile()
    return nc


def _host_pack(x, qkv_w, dw_w, proj_w, temperature, H):
    SH_ROWS = H + 2
    bfa = lambda a: np.ascontiguousarray(a.astype(ml_dtypes.bfloat16))
    wT = qkv_w.T.astype(np.float32)                     # [192, 576]
    dw9 = dw_w.reshape(576, 9).astype(np.float32)
    wdw = np.zeros((128, 45), np.float32)
    for j, (o0, orows) in enumerate(OT):
        wdw[0:orows, 9 * j:9 * j + 9] = dw9[o0:o0 + orows]
    pjT = proj_w.T.astype(np.float32)
    shared = {
        "wt1": bfa(wT[0:128]), "wt2": bfa(wT[128:192]), "wdw": wdw,
        "pjt1": bfa(pjT[0:128]), "pjt2": bfa(pjT[128:192]),
        "ident": bfa(np.eye(128, dtype=np.float32)),
        "tempb": np.ascontiguousarray(np.broadcast_to(
            np.asarray(temperature, np.float32).reshape(1, HEADS),
            (48, HEADS)).astype(np.float32)),
    }
    in_maps = []
    for core in range(N_CORES):
        xs = np.zeros((C, SH_ROWS, WP), np.float32)
        xs[:, 1:1 + H, 2:2 + W] = x[core]
        in_maps.append({**shared, "x": bfa(xs.reshape(C, SH_ROWS * WP))})
    return in_maps


def kernel(x, qkv_w, dw_w, proj_w, temperature, num_heads):
    x = np.asarray(x, np.float32)
    H = x.shape[2]
    assert int(num_heads) == HEADS and x.shape == (B, C, H, W)
    key = (H,)
    if key not in _BUILT:
        _BUILT[key] = build(H=H, CR=16 if H % 16 == 0 else H)
    nc = _BUILT[key]
    in_maps = _host_pack(x, np.asarray(qkv_w, np.float32),
                         np.asarray(dw_w, np.float32),
                         np.asarray(proj_w, np.float32),
                         np.asarray(temperature, np.float32).reshape(-1), H)
    res = run_bass_kernel_spmd(nc, in_maps, list(range(N_CORES)))
    CR = 16 if H % 16 == 0 else H
    NCH, XST, NV = H // CR, CR * WP, CR * W
    out = np.empty((B, C, H, W), np.float32)
    for core in range(N_CORES):
        yv = res.results[core]["y"].reshape(C, NCH, XST)[:, :, :NV]
        out[core] = yv.astype(np.float32).reshape(C, H, W)
    return out


def build_empty(H=256):
    """Same external IO as build(), trivial body — for launch-overhead calibration."""
    SH_ROWS = H + 2
    CR = 16 if H % 16 == 0 else H
    NCH, XST = H // CR, CR * WP
    nc = bacc.Bacc("TRN2", target_bir_lowering=False, debug=False,
                   num_devices=N_CORES)
    dram = lambda n, s, d, kind: nc.dram_tensor(n, s, d, kind=kind).ap()
    x_d = dram("x", [C, SH_ROWS * WP], bf16, "ExternalInput")
    dram("wt1", [128, 576], bf16, "ExternalInput")
    dram("wt2", [64, 576], bf16, "ExternalInput")
    dram("wdw", [128, 45], f32, "ExternalInput")
    dram("pjt1", [128, 192], bf16, "ExternalInput")
    dram("pjt2", [64, 192], bf16, "ExternalInput")
    dram("ident", [128, 128], bf16, "ExternalInput")
    dram("tempb", [48, 4], f32, "ExternalInput")
    dram("y", [C, NCH * XST], bf16, "ExternalOutput")
    dbg_d = dram("dbg", [128, 200], f32, "ExternalOutput")
    with tile.TileContext(nc) as tc, ExitStack() as ctx:
        sb = ctx.enter_context(tc.tile_pool(name="sb", bufs=1))
        t = sb.tile([128, 200], bf16)
        nc.sync.dma_start(t[:, 0:169], x_d[0:128, 0:169])
        t2 = sb.tile([128, 200], f32)
        nc.vector.tensor_copy(t2[:, 0:169], t[:, 0:169])
        nc.sync.dma_start(dbg_d[:, 0:169], t2[:, 0:169])
    nc.compile()
    return nc
